# revision 8
# baseline (speedup 1.0000x reference)
"""DeepseekV3 MoE kernel for 8 Trainium2 NeuronCores — top-4 sparse experts.

Sharding: expert-parallel (2 routed experts per core) + intermediate-sharded
shared expert (128 of 1024 columns per core); gate replicated in fp32 on
every core. Unlike the dense baseline, each expert only processes the
tokens routed to it: the combine weights are compacted on device
(sparse_gather), token activations are gathered with the hardware DGE
(dma_gather), expert MLPs run at a fixed capacity per half, and results are
scatter-added (gpsimd) into a dense accumulator initialized by the shared
expert. Two token halves pipeline compute against the per-half
ReduceScatter that combines partial outputs across cores.

Self-contained: hardcodes all shapes. Only dependency is the concourse
tree (on PYTHONPATH in the container) and numpy.
"""

import os
import sys

import numpy as np

for _p in ("/opt/trn_rl_repo", "/root/.axon_site/_ro/trn_rl_repo"):
    if os.path.isdir(_p) and _p not in sys.path:
        sys.path.append(_p)

import concourse.bacc as bacc
import concourse.mybir as mybir
import concourse.tile as tile
from concourse.bass_utils import run_bass_kernel_spmd
from concourse.masks import make_identity

F32 = mybir.dt.float32
F32R = mybir.dt.float32r
BF16 = mybir.dt.bfloat16
I16 = mybir.dt.int16
I32 = mybir.dt.int32
U32 = mybir.dt.uint32
AX = mybir.AxisListType.X
OP = mybir.AluOpType
ACT = mybir.ActivationFunctionType

H = 1024          # hidden size
M = 512           # expert intermediate
E = 16            # routed experts
EPC = 2           # experts per core
NCORES = 8
N = 2048          # tokens (B*S)
KT = H // 128     # 8 contraction tiles
MB = M // 128     # 4 m-tiles per expert
HT = H // 128     # 8 output h-tiles
SCALE = 2.5
SM = 128          # shared-expert intermediate columns per core
HALF = N // 2     # 1024 tokens per pipelined half
T = HALF // 128   # 8 token tiles per half


def _routing(nc, pool, s4c, comb):
    """Token-major DeepseekV3 noaux_tc routing for one [128, 16] tile.

    s4c: sigmoid(logits) + bias, [128, 16] fp32 SBUF.
    comb: output combine weights [128, 16] (SCALE * topk_weight scattered).
    """
    v = s4c.rearrange("p (g s) -> p g s", g=4)

    # sum of top-2 per group of 4 = max over the 6 pairwise sums
    pairs = pool.tile([128, 24], F32, tag="rt_pairs")
    pv = pairs.rearrange("p (g s) -> p g s", g=4)
    nc.vector.tensor_add(pv[:, :, 0:3], v[:, :, 0:3], v[:, :, 1:4])
    nc.vector.tensor_add(pv[:, :, 3:5], v[:, :, 0:2], v[:, :, 2:4])
    nc.vector.tensor_add(pv[:, :, 5:6], v[:, :, 0:1], v[:, :, 3:4])
    gsum = pool.tile([128, 4], F32, tag="rt_gsum")
    nc.vector.reduce_max(out=gsum, in_=pv, axis=AX)

    # 2nd largest group sum = max over the 6 pairwise mins
    gmins = pool.tile([128, 8], F32, tag="rt_gmins")
    nc.vector.tensor_tensor(gmins[:, 0:3], gsum[:, 0:3], gsum[:, 1:4], op=OP.min)
    nc.vector.tensor_tensor(gmins[:, 3:5], gsum[:, 0:2], gsum[:, 2:4], op=OP.min)
    nc.vector.tensor_tensor(gmins[:, 5:6], gsum[:, 0:1], gsum[:, 3:4], op=OP.min)
    t2g = pool.tile([128, 1], F32, tag="rt_t2g")
    nc.vector.reduce_max(out=t2g, in_=gmins[:, 0:6], axis=AX)

    # group mask (1.0 for the top-2 groups), expanded to 16 experts
    gmask = pool.tile([128, 4], F32, tag="rt_gmask")
    nc.vector.tensor_scalar(gmask, gsum, t2g, None, op0=OP.is_ge)
    mask16 = pool.tile([128, 16], F32, tag="rt_mask16")
    m16v = mask16.rearrange("p (g s) -> p g s", g=4)
    for j in range(4):
        nc.vector.tensor_copy(m16v[:, :, j], gmask)

    masked = pool.tile([128, 16], F32, tag="rt_masked")
    nc.vector.tensor_mul(masked, s4c, mask16)

    # top-4 of 16 via Max8, threshold select, normalize
    top8 = pool.tile([128, 8], F32, tag="rt_top8")
    nc.vector.max(out=top8, in_=masked)
    denom = pool.tile([128, 1], F32, tag="rt_denom")
    nc.vector.reduce_sum(out=denom, in_=top8[:, 0:4], axis=AX)
    w = pool.tile([128, 1], F32, tag="rt_w")
    nc.vector.tensor_scalar_add(denom, denom, 1e-20)
    nc.vector.reciprocal(w, denom)
    nc.vector.tensor_scalar_mul(w, w, SCALE)

    # sel_w = (masked >= t4) * w ; comb = sel_w * masked
    selw = pool.tile([128, 16], F32, tag="rt_selw")
    nc.vector.tensor_scalar(selw, masked, top8[:, 3:4], w, op0=OP.is_ge, op1=OP.mult)
    nc.vector.tensor_mul(comb, selw, masked)


def build_program(cap):
    cw16 = cap // 16
    nc = bacc.Bacc(
        "TRN2",
        target_bir_lowering=False,
        debug=False,
        enable_asserts=False,
        num_devices=NCORES,
    )

    xT = nc.dram_tensor("xT", [H, N], BF16, kind="ExternalInput").ap()
    xTf = nc.dram_tensor("xTf", [H, N], F32, kind="ExternalInput").ap()
    x_tok = nc.dram_tensor("x_tok", [N, H], BF16, kind="ExternalInput").ap()
    gk = nc.dram_tensor("gk", [H, E], F32, kind="ExternalInput").ap()
    gbr = nc.dram_tensor("gbr", [128, E], F32, kind="ExternalInput").ap()
    wg = nc.dram_tensor("wg", [EPC, H, M], BF16, kind="ExternalInput").ap()
    wu = nc.dram_tensor("wu", [EPC, H, M], BF16, kind="ExternalInput").ap()
    wd = nc.dram_tensor("wd", [EPC, M, H], BF16, kind="ExternalInput").ap()
    sg = nc.dram_tensor("sg", [H, SM], BF16, kind="ExternalInput").ap()
    su = nc.dram_tensor("su", [H, SM], BF16, kind="ExternalInput").ap()
    sd = nc.dram_tensor("sd", [SM, H], BF16, kind="ExternalInput").ap()
    sel2_in = nc.dram_tensor("sel2", [E, EPC], F32R, kind="ExternalInput").ap()
    selm_in = nc.dram_tensor("selm", [E, EPC * 128], F32R, kind="ExternalInput").ap()
    out = nc.dram_tensor("out", [128, N], BF16, kind="ExternalOutput").ap()

    with tile.TileContext(nc) as tc:
        with (
            tc.tile_pool(name="w", bufs=1) as wpool,
            tc.tile_pool(name="sb", bufs=2) as sb,
            tc.tile_pool(name="rt", bufs=2) as rt,
            tc.tile_pool(name="ps", bufs=2, space="PSUM") as ps,
            tc.tile_pool(name="dram", bufs=1, space="DRAM") as dram,
        ):
            # ---- gating-critical small DMAs first ----
            gk_sb = wpool.tile([128, KT * E], F32, tag="gk")
            for k in range(KT):
                nc.sync.dma_start(
                    out=gk_sb[:, k * E:(k + 1) * E],
                    in_=gk[k * 128:(k + 1) * 128, :],
                )
            gbr_sb = wpool.tile([128, E], F32, tag="gbr")
            nc.sync.dma_start(out=gbr_sb, in_=gbr)
            sel2_sb = wpool.tile([E, EPC], F32R, tag="sel2")
            nc.sync.dma_start(out=sel2_sb, in_=sel2_in)
            selm_sb = wpool.tile([E, EPC * 128], F32R, tag="selm")
            for e in range(EPC):
                nc.sync.dma_start(
                    out=selm_sb[:, e * 128:(e + 1) * 128],
                    in_=selm_in[:, e * 128:(e + 1) * 128],
                )
            ident = wpool.tile([128, 128], F32, tag="ident")
            make_identity(nc, ident)
            # iota over wrapped positions: iotaf1[p, j] = 16*j + p + 1
            iota_i = wpool.tile([16, HALF // 16], I32, tag="iota_i")
            nc.gpsimd.iota(iota_i, pattern=[[16, HALF // 16]], base=0,
                           channel_multiplier=1)
            iotaf1 = wpool.tile([16, HALF // 16], F32, tag="iotaf1")
            nc.vector.tensor_copy(iotaf1, iota_i)
            nc.vector.tensor_scalar_add(iotaf1, iotaf1, 1.0)
            # wrapped slot positions [16, cw16] as f32, for tail masking
            pos_i = wpool.tile([16, cw16], I32, tag="pos_i")
            nc.gpsimd.iota(pos_i, pattern=[[16, cw16]], base=0,
                           channel_multiplier=1)
            posf = wpool.tile([16, cw16], F32, tag="posf")
            nc.vector.tensor_copy(posf, pos_i)

            def emit_act_dma(h):
                off = h * HALF
                xtf_t = sb.tile([128, KT * HALF], F32, tag="xtf", bufs=1)
                for k in range(KT):
                    nc.sync.dma_start(
                        out=xtf_t[:, k * HALF:(k + 1) * HALF],
                        in_=xTf[k * 128:(k + 1) * 128, off:off + HALF],
                    )
                xt_t = sb.tile([128, KT * HALF], BF16, tag="xt", bufs=1)
                for k in range(KT):
                    nc.sync.dma_start(
                        out=xt_t[:, k * HALF:(k + 1) * HALF],
                        in_=xT[k * 128:(k + 1) * 128, off:off + HALF],
                    )
                return xt_t, xtf_t

            acts = {0: emit_act_dma(0)}

            # ---- resident weights (after chunk-0 activations) ----
            wg_sb = []
            wu_sb = []
            wd_sb = []
            for e in range(EPC):
                g_t = wpool.tile([128, KT * M], BF16, name=f"wg_sb{e}", tag=f"wg{e}")
                u_t = wpool.tile([128, KT * M], BF16, name=f"wu_sb{e}", tag=f"wu{e}")
                for k in range(KT):
                    nc.sync.dma_start(
                        out=g_t[:, k * M:(k + 1) * M],
                        in_=wg[e, k * 128:(k + 1) * 128, :],
                    )
                    nc.sync.dma_start(
                        out=u_t[:, k * M:(k + 1) * M],
                        in_=wu[e, k * 128:(k + 1) * 128, :],
                    )
                wg_sb.append(g_t)
                wu_sb.append(u_t)

            sg_sb = wpool.tile([128, KT * SM], BF16, tag="sg")
            su_sb = wpool.tile([128, KT * SM], BF16, tag="su")
            for k in range(KT):
                nc.sync.dma_start(
                    out=sg_sb[:, k * SM:(k + 1) * SM],
                    in_=sg[k * 128:(k + 1) * 128, :],
                )
                nc.sync.dma_start(
                    out=su_sb[:, k * SM:(k + 1) * SM],
                    in_=su[k * 128:(k + 1) * 128, :],
                )

            for e in range(EPC):
                d_t = wpool.tile([128, MB * H], BF16, name=f"wd_sb{e}", tag=f"wd{e}")
                for mb in range(MB):
                    nc.sync.dma_start(
                        out=d_t[:, mb * H:(mb + 1) * H],
                        in_=wd[e, mb * 128:(mb + 1) * 128, :],
                    )
                wd_sb.append(d_t)
            sd_sb = wpool.tile([128, H], BF16, tag="sd")
            nc.sync.dma_start(out=sd_sb, in_=sd)

            def emit_gating(h, xtf_t):
                """fp32 logits + routing; returns (combT, mycw_all)."""
                combT = sb.tile([E, HALF], F32R, tag="combT", bufs=1)
                mycw_all = sb.tile([128, EPC * T], F32, tag="mycw_all")
                for t in range(T):
                    plt = ps.tile([128, E], F32, tag="pmisc",
                                  padded_shape=[128, 512])
                    for k in range(KT):
                        nc.tensor.matmul(
                            plt,
                            lhsT=xtf_t[:, k * HALF + t * 128:
                                       k * HALF + (t + 1) * 128],
                            rhs=gk_sb[:, k * E:(k + 1) * E],
                            start=(k == 0),
                            stop=(k == KT - 1),
                        )
                    s4c = rt.tile([128, E], F32, tag="rt_s4c")
                    nc.scalar.activation(s4c, plt, ACT.Sigmoid)
                    nc.vector.tensor_add(s4c, s4c, gbr_sb)
                    comb = rt.tile([128, E], F32, tag="rt_comb", bufs=8)
                    _routing(nc, rt, s4c, comb)
                    pct = ps.tile([E, 128], F32, tag="pmisc",
                                  padded_shape=[128, 512])
                    nc.tensor.transpose(pct, comb, ident)
                    nc.scalar.copy(combT[:, t * 128:(t + 1) * 128], pct)
                    # my experts' combine columns: [128 tok, EPC]
                    pcw = ps.tile([128, EPC], F32, tag="pmisc",
                                  padded_shape=[128, 512])
                    nc.tensor.matmul(
                        pcw,
                        lhsT=combT[:, t * 128:(t + 1) * 128],
                        rhs=sel2_sb,
                        start=True,
                        stop=True,
                    )
                    nc.scalar.copy(
                        mycw_all.rearrange("p (e t) -> p e t", e=EPC)[:, :, t],
                        pcw,
                    )
                return combT, mycw_all

            def emit_cbc(h, combT):
                """dense per-token weight, bcast over partitions, per expert."""
                cbc = []
                for le in range(EPC):
                    cb = sb.tile([128, HALF], F32, tag=f"cbc{le}", bufs=1)
                    for wb in range(2):
                        pb = ps.tile([128, 512], F32, tag="pga")
                        nc.tensor.matmul(
                            pb,
                            lhsT=selm_sb[:, le * 128:(le + 1) * 128],
                            rhs=combT[:, wb * 512:(wb + 1) * 512],
                            start=True,
                            stop=True,
                        )
                        nc.scalar.copy(cb[:, wb * 512:(wb + 1) * 512], pb)
                    cbc.append(cb)
                return cbc

            def emit_compact(h, mycw_all):
                """wrapped compaction -> (sid_rep, gid_rep) per expert."""
                reps = []
                for le in range(EPC):
                    wcw = sb.tile([16, HALF // 16], F32, tag=f"wcw{le}")
                    wcw_v = wcw.rearrange("p (t u) -> p t u", u=8)
                    for u in range(8):
                        nc.sync.dma_start(
                            out=wcw_v[:, :, u],
                            in_=mycw_all[u * 16:(u + 1) * 16,
                                         le * T:(le + 1) * T],
                        )
                    m = sb.tile([16, HALF // 16], F32, tag=f"m{le}")
                    nc.vector.tensor_scalar(m, wcw, 0.0, None, op0=OP.is_gt)
                    wid = sb.tile([16, HALF // 16], F32, tag=f"wid{le}")
                    nc.vector.tensor_tensor(wid, m, iotaf1, op=OP.mult)
                    nc.vector.tensor_scalar_add(wid, wid, -1.0)
                    idw = sb.tile([16, cw16], F32, tag=f"idw{le}")
                    nf = sb.tile([1, 1], U32, tag=f"nf{le}")
                    nc.gpsimd.sparse_gather(idw, wid, num_found=nf)
                    # HW writes garbage past num_found: mask the tail
                    nff = sb.tile([1, 1], F32, tag=f"nff{le}")
                    nc.vector.tensor_copy(nff, nf)
                    nfb = sb.tile([16, 1], F32, tag=f"nfb{le}")
                    nc.gpsimd.partition_broadcast(nfb, nff)
                    validf = sb.tile([16, cw16], F32, tag=f"validf{le}")
                    nc.vector.tensor_scalar(validf, posf, nfb, None,
                                            op0=OP.is_lt)
                    valid = sb.tile([16, cw16], I16, tag=f"valid{le}")
                    nc.vector.tensor_copy(valid, validf)
                    idw16 = sb.tile([16, cw16], I16, tag=f"idw16{le}")
                    nc.vector.tensor_copy(idw16, idw)
                    sid16 = sb.tile([16, cw16], I16, tag=f"sid16{le}")
                    nc.vector.tensor_scalar_add(idw16, idw16, 1)
                    nc.vector.tensor_tensor(idw16, idw16, valid, op=OP.mult)
                    nc.vector.tensor_scalar_add(sid16, idw16, -1)
                    gid16 = sb.tile([16, cw16], I16, tag=f"gid16{le}")
                    nc.vector.tensor_scalar(gid16, sid16, 0, None, op0=OP.max)
                    sid_rep = sb.tile([128, cw16], I16, tag=f"sidrep{le}")
                    gid_rep = sb.tile([128, cw16], I16, tag=f"gidrep{le}")
                    for g in range(8):
                        nc.sync.dma_start(
                            out=sid_rep[g * 16:(g + 1) * 16, :], in_=sid16)
                        nc.sync.dma_start(
                            out=gid_rep[g * 16:(g + 1) * 16, :], in_=gid16)
                    reps.append((sid_rep, gid_rep))
                return reps

            def emit_gather(h, le, gid_rep, cbc_le):
                xg = sb.tile([128, KT * cap], BF16, tag=f"xg{le}")
                nc.gpsimd.dma_gather(
                    out_ap=xg.rearrange("p (k c) -> p k c", k=KT),
                    in_ap=x_tok[h * HALF:(h + 1) * HALF, :],
                    idxs_ap=gid_rep,
                    num_idxs=cap,
                    num_idxs_reg=cap,
                    elem_size=H,
                    transpose=True,
                )
                cwb = sb.tile([128, cap], F32, tag=f"cwb{le}")
                nc.gpsimd.ap_gather(
                    out_ap=cwb,
                    in_ap=cbc_le,
                    idxs_ap=gid_rep,
                    channels=128,
                    num_elems=HALF,
                    d=1,
                    num_idxs=cap,
                )
                return xg, cwb

            def emit_shared(h, xt_t, yacc):
                """shared expert for the half; initializes yacc."""
                yacc_v = yacc.rearrange("p (n d) -> p n d", d=HT)
                inter_s = sb.tile([128, HALF], BF16, tag="inter_s")
                for wb in range(2):
                    pgs = ps.tile([128, 512], F32, tag="pga")
                    pus = ps.tile([128, 512], F32, tag="pub")
                    for k in range(KT):
                        nc.tensor.matmul(
                            pgs,
                            lhsT=sg_sb[:, k * SM:(k + 1) * SM],
                            rhs=xt_t[:, k * HALF + wb * 512:
                                     k * HALF + (wb + 1) * 512],
                            start=(k == 0),
                            stop=(k == KT - 1),
                        )
                    for k in range(KT):
                        nc.tensor.matmul(
                            pus,
                            lhsT=su_sb[:, k * SM:(k + 1) * SM],
                            rhs=xt_t[:, k * HALF + wb * 512:
                                     k * HALF + (wb + 1) * 512],
                            start=(k == 0),
                            stop=(k == KT - 1),
                        )
                    sig_s = sb.tile([128, 512], BF16, tag="sig")
                    nc.scalar.activation(sig_s, pgs, ACT.Sigmoid)
                    sgs = sb.tile([128, 512], BF16, tag="silu")
                    nc.vector.scalar_tensor_tensor(
                        sgs, pgs, 1.0, sig_s, op0=OP.mult, op1=OP.mult
                    )
                    nc.vector.tensor_mul(
                        inter_s[:, wb * 512:(wb + 1) * 512], sgs, pus)
                for ht in range(HT):
                    for wb in range(2):
                        pys = ps.tile([128, 512], F32, tag="py")
                        nc.tensor.matmul(
                            pys,
                            lhsT=sd_sb[:, ht * 128:(ht + 1) * 128],
                            rhs=inter_s[:, wb * 512:(wb + 1) * 512],
                            start=True,
                            stop=True,
                        )
                        nc.scalar.copy(
                            yacc_v[:, wb * 512:(wb + 1) * 512, ht], pys)

            def emit_expert_gu(h, le, xg, cwb):
                """g/u projections + inter = silu(g) * (u * combine)."""
                xg_v = xg.rearrange("p (k c) -> p k c", k=KT)
                it = sb.tile([128, MB * cap], BF16, tag=f"inter{le}", bufs=1)
                for mb in range(MB):
                    pg = ps.tile([128, cap], F32, tag="pga",
                                 padded_shape=[128, 512])
                    for k in range(KT):
                        nc.tensor.matmul(
                            pg,
                            lhsT=wg_sb[le][:, k * M + mb * 128:
                                           k * M + (mb + 1) * 128],
                            rhs=xg_v[:, k, :],
                            start=(k == 0),
                            stop=(k == KT - 1),
                        )
                    pu = ps.tile([128, cap], F32, tag="pub",
                                 padded_shape=[128, 512])
                    for k in range(KT):
                        nc.tensor.matmul(
                            pu,
                            lhsT=wu_sb[le][:, k * M + mb * 128:
                                           k * M + (mb + 1) * 128],
                            rhs=xg_v[:, k, :],
                            start=(k == 0),
                            stop=(k == KT - 1),
                        )
                    sig_t = sb.tile([128, cap], BF16, tag="sig",
                                    padded_shape=[128, 512])
                    nc.scalar.activation(sig_t, pg, ACT.Sigmoid)
                    sg_t = sb.tile([128, cap], BF16, tag="silu",
                                   padded_shape=[128, 512])
                    nc.vector.scalar_tensor_tensor(
                        sg_t, pg, 1.0, sig_t, op0=OP.mult, op1=OP.mult
                    )
                    us = sb.tile([128, cap], BF16, tag="us",
                                 padded_shape=[128, 512])
                    nc.vector.tensor_mul(us, pu, cwb)
                    nc.vector.tensor_mul(it[:, mb * cap:(mb + 1) * cap],
                                         sg_t, us)
                return it

            def emit_expert_down(h, le, it, sid_rep, yacc):
                addbuf = sb.tile([128, cap * HT], BF16, tag=f"addbuf{le}",
                                 bufs=1)
                ab_v = addbuf.rearrange("p (c d) -> p c d", d=HT)
                for ht in range(HT):
                    py = ps.tile([128, cap], F32, tag="py",
                                 padded_shape=[128, 512])
                    for mb in range(MB):
                        nc.tensor.matmul(
                            py,
                            lhsT=wd_sb[le][:, mb * H + ht * 128:
                                           mb * H + (ht + 1) * 128],
                            rhs=it[:, mb * cap:(mb + 1) * cap],
                            start=(mb == 0),
                            stop=(mb == MB - 1),
                        )
                    nc.scalar.copy(ab_v[:, :, ht], py)
                nc.gpsimd.scatter_add(
                    in_ap=yacc.rearrange("p (n d) -> p n d", d=HT),
                    idxs_ap=sid_rep,
                    add_ap=ab_v,
                    channels=128,
                    num_elems=HALF,
                    d=HT,
                    num_idxs=cap,
                )

            def emit_out(h, yacc):
                yacc_v = yacc.rearrange("p (n d) -> p n d", d=HT)
                ypart = dram.tile([H, HALF], BF16, name=f"ypart{h}",
                                  tag=f"ypart{h}")
                for ht in range(HT):
                    ypT = sb.tile([128, HALF], BF16, tag="ypT")
                    nc.vector.tensor_copy(ypT, yacc_v[:, :, ht])
                    nc.sync.dma_start(
                        out=ypart[ht * 128:(ht + 1) * 128, :], in_=ypT)
                rs_out = dram.tile([128, HALF], BF16, name=f"rsout{h}",
                                   tag=f"rsout{h}")
                nc.gpsimd.collective_compute(
                    "ReduceScatter",
                    OP.add,
                    replica_groups=[list(range(NCORES))],
                    ins=[ypart.opt()],
                    outs=[rs_out.opt()],
                )
                nc.gpsimd.dma_start(out=out[:, h * HALF:(h + 1) * HALF],
                                    in_=rs_out)

            # ================= pipelined emission =================
            # half 0 front-end
            combT0, mycw0 = emit_gating(0, acts[0][1])
            cbc0 = emit_cbc(0, combT0)
            reps0 = emit_compact(0, mycw0)
            g0 = [emit_gather(0, le, reps0[le][1], cbc0[le])
                  for le in range(EPC)]
            acts[1] = emit_act_dma(1)

            yacc0 = sb.tile([128, HALF * HT], BF16, tag="yacc", bufs=1,
                            name="yacc0")
            emit_shared(0, acts[0][0], yacc0)
            it00 = emit_expert_gu(0, 0, g0[0][0], g0[0][1])
            # half-1 gating overlaps half-0 expert compute
            combT1, mycw1 = emit_gating(1, acts[1][1])
            it01 = emit_expert_gu(0, 1, g0[1][0], g0[1][1])
            emit_expert_down(0, 0, it00, reps0[0][0], yacc0)
            emit_expert_down(0, 1, it01, reps0[1][0], yacc0)

            cbc1 = emit_cbc(1, combT1)
            reps1 = emit_compact(1, mycw1)
            g1 = [emit_gather(1, le, reps1[le][1], cbc1[le])
                  for le in range(EPC)]

            emit_out(0, yacc0)

            yacc1 = sb.tile([128, HALF * HT], BF16, tag="yacc", bufs=1,
                            name="yacc1")
            emit_shared(1, acts[1][0], yacc1)
            it10 = emit_expert_gu(1, 0, g1[0][0], g1[0][1])
            it11 = emit_expert_gu(1, 1, g1[1][0], g1[1][1])
            emit_expert_down(1, 0, it10, reps1[0][0], yacc1)
            emit_expert_down(1, 1, it11, reps1[1][0], yacc1)
            emit_out(1, yacc1)

    nc.compile()
    return nc


_NC_CACHE = {}


def _get_program(cap):
    if cap not in _NC_CACHE:
        _NC_CACHE[cap] = build_program(cap)
    return _NC_CACHE[cap]


def _host_expert_counts(x, gk, gb):
    """numpy replica of the gate, for capacity sizing only."""
    logits = x @ gk
    scores = 1.0 / (1.0 + np.exp(-logits))
    s4c = scores + gb[None, :]
    n = x.shape[0]
    gsc = s4c.reshape(n, 4, 4)
    top2 = np.sort(gsc, axis=-1)[:, :, 2:].sum(-1)
    thr = np.sort(top2, axis=-1)[:, 2:3]
    gmask = (top2 >= thr).astype(np.float32)
    masked = s4c * np.repeat(gmask, 4, axis=1)
    t4 = np.sort(masked, axis=-1)[:, -4:-3]
    sel = masked >= t4
    counts = np.zeros((2, E), dtype=np.int64)
    half = n // 2
    counts[0] = sel[:half].sum(0)
    counts[1] = sel[half:].sum(0)
    return counts


def _make_in_maps(inputs):
    import ml_dtypes
    bf16 = ml_dtypes.bfloat16
    x = np.asarray(inputs["hidden_states"], dtype=np.float32).reshape(N, H)
    xT_f = np.ascontiguousarray(x.T)
    xT_bf = xT_f.astype(bf16)
    x_tok = np.ascontiguousarray(x).astype(bf16)
    gk = np.ascontiguousarray(np.asarray(inputs["gate_kernel"], dtype=np.float32))
    gb = np.asarray(inputs["gate_bias"], dtype=np.float32)
    gbr = np.ascontiguousarray(np.broadcast_to(gb[None, :], (128, E)))
    w_gate = np.asarray(inputs["w_gate"], dtype=np.float32)
    w_up = np.asarray(inputs["w_up"], dtype=np.float32)
    w_down = np.asarray(inputs["w_down"], dtype=np.float32)
    sw_gate = np.asarray(inputs["sw_gate"], dtype=np.float32)
    sw_up = np.asarray(inputs["sw_up"], dtype=np.float32)
    sw_down = np.asarray(inputs["sw_down"], dtype=np.float32)

    counts = _host_expert_counts(x, gk, gb)
    maxc = int(counts.max())
    cap = max(384, -(-maxc // 128) * 128)

    in_maps = []
    for c in range(NCORES):
        sel2 = np.zeros((E, EPC), dtype=np.float32)
        selm = np.zeros((E, EPC * 128), dtype=np.float32)
        for e in range(EPC):
            sel2[EPC * c + e, e] = 1.0
            selm[EPC * c + e, e * 128:(e + 1) * 128] = 1.0
        in_maps.append({
            "xT": xT_bf,
            "xTf": xT_f,
            "x_tok": x_tok,
            "gk": gk,
            "gbr": gbr,
            "wg": np.ascontiguousarray(w_gate[EPC * c:EPC * (c + 1)]).astype(bf16),
            "wu": np.ascontiguousarray(w_up[EPC * c:EPC * (c + 1)]).astype(bf16),
            "wd": np.ascontiguousarray(w_down[EPC * c:EPC * (c + 1)]).astype(bf16),
            "sg": np.ascontiguousarray(sw_gate[:, SM * c:SM * (c + 1)]).astype(bf16),
            "su": np.ascontiguousarray(sw_up[:, SM * c:SM * (c + 1)]).astype(bf16),
            "sd": np.ascontiguousarray(sw_down[SM * c:SM * (c + 1), :]).astype(bf16),
            "sel2": sel2,
            "selm": selm,
        })
    return in_maps, cap


def run(inputs, trace=False):
    """Returns (output, BassKernelResults)."""
    in_maps, cap = _make_in_maps(inputs)
    nc = _get_program(cap)
    res = run_bass_kernel_spmd(
        nc, in_maps, core_ids=list(range(NCORES)), trace=trace
    )
    yT = np.concatenate(
        [np.asarray(res.results[c]["out"], dtype=np.float32) for c in range(NCORES)],
        axis=0,
    )
    y = np.ascontiguousarray(yT.T).reshape(2, 1024, H).astype(np.float32)
    return y, res


def kernel(**inputs):
    y, _ = run(inputs, trace=False)
    return y
